# revision 9
# baseline (speedup 1.0000x reference)
"""Depth-to-space (pixel shuffle / DUC) kernel for Trainium2.

Full op: x[16, 1216, 32, 32] f32 -> out[16, 19, 304, 304] f32 where
  out[b, c, i*8+r1, j*8+r2] = x[b, c*64 + r1*8 + r2, i, j]
and out is zero-padded from 256x256 up to 304x304 (bottom/right).

Sharding: pure data-parallel over batch, 2 examples per core on 8 cores.

Pure data movement (~10 MB in + ~10 MB out per core); the HBM roofline
(716 GB/s per stack shared by 2 NCs -> ~358 GB/s/NC) puts the floor at
~56 us for the ~20 MB round trip, so the kernel is organized as a
software pipeline that keeps load and store DMA streaming concurrently
with the on-chip shuffle:

  Per class c (19 total), three ops on three different queues:
    load   (sync HWDGE ring)   one 128-partition DMA, 4KB descriptors
           p = img*64 + u2 (u2 = r1*8+r2), free = i*32 + j
    shuffle (DVE)              one stream-transpose of [128, 1024]:
           32x32 logical squares exchange partition-low5 (r1l,r2) with
           i; in/out APs fold the remaining free reorders.
    store  (scalar HWDGE ring) one 128-partition DMA, 1KB descriptors
           p = img*64 + s*32 + i  ->  out rows i*8 + s*4 + r1l,
           cols (j, r2) contiguous 0:256.

Loads live on one HWDGE ring and stores on the other so a store's
semaphore wait (for its transpose) never head-of-line-blocks descriptor
generation for later loads.  tile_pool(bufs=N) gives the Tile scheduler
N chunks of lookahead, so in steady state the 16 SDMA engines see load
and store packets continuously (round-robin at packet granularity) and
the 19 DVE transposes (~18 us total) hide entirely under the ~50+ us of
DMA.  Zero padding relies on ExternalOutput buffers being pre-zeroed by
the runner (both the native path and the PJRT/axon path guarantee this).
"""

import sys

if "/opt/trn_rl_repo" not in sys.path:
    sys.path.insert(0, "/opt/trn_rl_repo")

import numpy as np

B, CLASSES, R, H, W, OUT = 16, 19, 8, 32, 32, 304
HR = H * R  # 256
N_CORES = 8
BPC = B // N_CORES  # batches per core = 2

_NC_CACHE = {}


def build_nc(
    bpc=BPC,
    classes=CLASSES,
    num_devices=N_CORES,
    loop_repeats=1,
    chunk=1,
    bufs=6,
    load_ring="sync",
    store_ring="scalar",
    merged_store=True,
    padded_store=False,
):
    import concourse.bacc as bacc
    import concourse.mybir as mybir
    from concourse.tile import TileContext

    f32 = mybir.dt.float32
    # Bacc (not plain Bass): its compile() legalizes multi-sem sync waits
    # that walrus otherwise rejects ("Too many sync wait commands").
    nc = bacc.Bacc(
        "TRN2", target_bir_lowering=False, debug=False, num_devices=num_devices
    )
    x = nc.declare_dram_parameter("x", [bpc, classes * R * R, H, W], f32, isOutput=False)
    out = nc.declare_dram_parameter("out", [bpc, classes, OUT, OUT], f32, isOutput=True)

    bounds = [(k, min(k + chunk, classes)) for k in range(0, classes, chunk)]

    if padded_store:
        # V2: store full 304-wide rows (pad cols pre-zeroed in SBUF) so each
        # (img, class) store is ONE DMA over a contiguous 304KB DRAM range
        # (256 rows x 1216B).  +19% store bytes for 2x fewer store DMAs with
        # 4.8x bigger descriptors.  Row slot layout per class per partition:
        # f = r1l*304 + j*8 + r2, f in [r1l*304+256, r1l*304+304) stays 0.
        with TileContext(nc) as tc:
            with tc.tile_pool(name="rawp", bufs=bufs) as rawp, \
                 tc.tile_pool(name="rowp", bufs=bufs) as rowp:

                def _body():
                    xs = x[:].rearrange(
                        "img (c u2) i j -> img u2 c (i j)", u2=R * R
                    )
                    for c in range(classes):
                        raw = rawp.tile([128, 1024], f32, tag="raw")
                        getattr(nc, load_ring).dma_start(
                            out=raw[:], in_=xs[:, :, c : c + 1]
                        )
                        row = rowp.tile([128, 4 * OUT], f32, tag="row")
                        rq = row[:].rearrange("p (r1l q) -> p r1l q", r1l=4)
                        # pad cols: tiny strided memset, on GpSimd so DVE
                        # and the HWDGE ring sequencers stay free
                        nc.gpsimd.memset(rq[:, :, HR:OUT], 0.0)
                        s_v = raw[:].rearrange("p (i j) -> p j i", i=H)
                        d_v = rq[:, :, 0:HR].rearrange(
                            "p r1l (j r2) -> p j r1l r2", r2=8
                        )
                        nc.vector.transpose(d_v, s_v)
                        src_v = row[:].rearrange(
                            "(img s2 i) f -> img i s2 f", img=bpc, s2=2
                        )
                        for img in range(bpc):
                            ring = (
                                ("scalar", "sync")[img]
                                if store_ring == "split"
                                else store_ring
                            )
                            getattr(nc, ring).dma_start(
                                out=out[img, c, 0:HR, :], in_=src_v[img]
                            )

                if loop_repeats > 1:
                    with tc.For_i(0, loop_repeats, 1):
                        _body()
                else:
                    _body()
        nc.compile()
        return nc

    with TileContext(nc) as tc:
        with tc.tile_pool(name="rawp", bufs=bufs) as rawp, \
             tc.tile_pool(name="rowp", bufs=bufs) as rowp:

          def _body():
            # DRAM-side load AP: (img, u2) together span the 128 partitions
            # (positional stream correspondence; shapes need not match).
            xs = x[:].rearrange("img (c u2) i j -> img u2 c (i j)", u2=R * R)
            for c0, c1 in bounds:
                w = c1 - c0
                raw = rawp.tile([128, w * 1024], f32, tag="raw")
                row = rowp.tile([128, w * 1024], f32, tag="row")
                # ---- load: one 128-partition DMA, 4KB DRAM descriptors ----
                getattr(nc, load_ring).dma_start(
                    out=raw[:], in_=xs[:, :, c0:c1]
                )
                # ---- shuffle: one DVE stream-transpose per class ----
                s_v = raw[:].rearrange("p (c i j) -> p c j i", c=w, i=H)
                d_v = row[:].rearrange(
                    "p (c r1l j r2) -> p c j r1l r2", c=w, r1l=4, r2=8
                )
                for cc in range(w):
                    nc.vector.transpose(d_v[:, cc], s_v[:, cc])
                # ---- store: one DMA per (class, img, s); 1KB DRAM rows ----
                # (3-dim DMA AP limit + the transpose's fixed partition
                # layout img*64+s*32+i make this the coarsest legal split.)
                row_s = row[:].rearrange(
                    "(img s i) (c r1l w2) -> img s c i r1l w2",
                    img=bpc, s=2, c=w, r1l=4,
                )
                for cc in range(w):
                    for img in range(bpc):
                        dst = out[img, c0 + cc, 0:HR, 0:HR].rearrange(
                            "(i s2 r1l) w2 -> s2 i r1l w2", s2=2, r1l=4
                        )
                        for s in range(2):
                            getattr(nc, store_ring).dma_start(
                                out=dst[s], in_=row_s[img, s, cc]
                            )

          if loop_repeats > 1:
              # measurement-only: on-device loop to amortize dispatch noise
              with tc.For_i(0, loop_repeats, 1):
                  _body()
          else:
              _body()
    nc.compile()
    return nc


def _get_nc():
    key = "main"
    if key not in _NC_CACHE:
        _NC_CACHE[key] = build_nc()
    return _NC_CACHE[key]


def kernel(x: np.ndarray) -> np.ndarray:
    from concourse.bass_utils import run_bass_kernel_spmd

    x = np.ascontiguousarray(x, dtype=np.float32)
    assert x.shape == (B, CLASSES * R * R, H, W), x.shape
    nc = _get_nc()
    in_maps = [{"x": x[k * BPC : (k + 1) * BPC]} for k in range(N_CORES)]
    res = run_bass_kernel_spmd(nc, in_maps, list(range(N_CORES)))
    return np.concatenate([res.results[k]["out"] for k in range(N_CORES)], axis=0)


# revision 19
# speedup vs baseline: 3.2616x; 3.2616x over previous
"""Depth-to-space (pixel shuffle / DUC) kernel for Trainium2.

Full op: x[16, 1216, 32, 32] f32 -> out[16, 19, 304, 304] f32 where
  out[b, c, i*8+r1, j*8+r2] = x[b, c*64 + r1*8 + r2, i, j]
and out is zero-padded from 256x256 up to 304x304 (bottom/right).

Sharding: pure data-parallel over batch, 2 examples per core on 8 cores.

Pure data movement (~10 MB in + ~10 MB out per core); the HBM roofline
(716 GB/s per stack shared by 2 NCs -> ~358 GB/s/NC) puts the floor at
~56 us for the ~20 MB round trip, so the kernel is organized as a
software pipeline that keeps load and store DMA streaming concurrently
with the on-chip shuffle:

  Per class c (19 total), three ops on three different queues:
    load   (sync HWDGE ring)   one 128-partition DMA, 4KB descriptors
           p = img*64 + u2 (u2 = r1*8+r2), free = i*32 + j
    shuffle (DVE)              one stream-transpose of [128, 1024]:
           32x32 logical squares exchange partition-low5 (r1l,r2) with
           i; in/out APs fold the remaining free reorders.
    store  (scalar HWDGE ring) one 128-partition DMA, 1KB descriptors
           p = img*64 + s*32 + i  ->  out rows i*8 + s*4 + r1l,
           cols (j, r2) contiguous 0:256.

Loads live on one HWDGE ring and stores on the other so a store's
semaphore wait (for its transpose) never head-of-line-blocks descriptor
generation for later loads.  tile_pool(bufs=N) gives the Tile scheduler
N chunks of lookahead, so in steady state the 16 SDMA engines see load
and store packets continuously (round-robin at packet granularity) and
the 19 DVE transposes (~18 us total) hide entirely under the ~50+ us of
DMA.  Zero padding relies on ExternalOutput buffers being pre-zeroed by
the runner (both the native path and the PJRT/axon path guarantee this).
"""

import sys

if "/opt/trn_rl_repo" not in sys.path:
    sys.path.insert(0, "/opt/trn_rl_repo")

import numpy as np

B, CLASSES, R, H, W, OUT = 16, 19, 8, 32, 32, 304
HR = H * R  # 256
N_CORES = 8
BPC = B // N_CORES  # batches per core = 2

_NC_CACHE = {}


def build_nc(
    bpc=BPC,
    classes=CLASSES,
    num_devices=N_CORES,
    loop_repeats=1,
    chunk=1,
    bufs=6,
    load_ring="sync",
    store_ring="scalar",
    merged_store=True,
    padded_store=False,
    group_count=0,
    ring_split=False,
    s6_groups=0,
    merged_dve=False,
    swdge_stores=0,
    v0_mode=True,
    load_chunks=4,
    load_desc_split=1,
):
    import concourse.bacc as bacc
    import concourse.mybir as mybir
    from concourse.tile import TileContext

    f32 = mybir.dt.float32
    # Bacc (not plain Bass): its compile() legalizes multi-sem sync waits
    # that walrus otherwise rejects ("Too many sync wait commands").
    nc = bacc.Bacc(
        "TRN2", target_bir_lowering=False, debug=False, num_devices=num_devices
    )
    x = nc.declare_dram_parameter("x", [bpc, classes * R * R, H, W], f32, isOutput=False)
    out = nc.declare_dram_parameter("out", [bpc, classes, OUT, OUT], f32, isOutput=True)

    if v0_mode and not (group_count or padded_store or s6_groups):
        # Proven-fastest structure (~76us/rep measured): one big raw/row
        # tile pair, subtile (range) deps let loads -> transposes -> stores
        # chase each other; every phase's DMAs split across both HWDGE
        # rings by img (img0=sync/even engines, img1=scalar/odd engines).
        FREE = classes * 1024
        step = (classes + load_chunks - 1) // load_chunks
        lbounds = [(k * step, min((k + 1) * step, classes))
                   for k in range(load_chunks)]
        lbounds = [(a, b) for a, b in lbounds if a < b]
        ring = ("sync", "scalar")

        with TileContext(nc) as tc:
            with tc.tile_pool(name="buf", bufs=1) as pool:

                def _body():
                    raw = pool.tile([128, FREE], f32, tag="raw", name="raw")
                    row = pool.tile([128, FREE], f32, tag="row", name="row")
                    raw_l = raw[:].rearrange(
                        "(img u2) (c v) -> img u2 c v", img=bpc, c=classes
                    )
                    for img in range(bpc):
                        if load_desc_split > 1:
                            xs = x[img].rearrange(
                                "(c u2) (q r) j -> u2 c q (r j)",
                                u2=R * R, q=load_desc_split,
                            )
                        else:
                            xs = x[img].rearrange(
                                "(c u2) i j -> u2 c (i j)", u2=R * R
                            )
                        for c0, c1 in lbounds:
                            getattr(nc, ring[img]).dma_start(
                                out=raw_l[img, :, c0:c1], in_=xs[:, c0:c1]
                            )
                    s_v = raw[:].rearrange("p (c i j) -> p c j i",
                                           c=classes, i=H)
                    d_v = row[:].rearrange(
                        "p (c r1l j r2) -> p c j r1l r2",
                        c=classes, r1l=4, r2=8,
                    )
                    for c in range(classes):
                        nc.vector.transpose(d_v[:, c], s_v[:, c])
                    row_s = row[:].rearrange(
                        "(img s i) (c r1l w2) -> img s c i r1l w2",
                        img=bpc, s=2, c=classes, r1l=4,
                    )
                    for c in range(classes):
                        for img in range(bpc):
                            dst = out[img, c, 0:HR, 0:HR].rearrange(
                                "(i s2 r1l) w2 -> s2 i r1l w2", s2=2, r1l=4
                            )
                            for s in range(2):
                                getattr(nc, ring[img]).dma_start(
                                    out=dst[s], in_=row_s[img, s, c]
                                )

                if loop_repeats > 1:
                    with tc.For_i(0, loop_repeats, 1):
                        _body()
                else:
                    _body()
        nc.compile()
        return nc

    bounds = [(k, min(k + chunk, classes)) for k in range(0, classes, chunk)]

    if group_count:
        # S3: coarse-grained software pipeline.  Classes split into
        # group_count groups; per group one load DMA per img (long free
        # runs on the sync ring), a burst of DVE transposes, and a burst
        # of stores on the scalar ring.  Waits per ring scale with
        # group_count, not DMA count, so HWDGE rings keep long
        # uninterrupted runs while load(g+1) overlaps store(g-1).
        gsz = (classes + group_count - 1) // group_count
        gbounds = [
            (k, min(k + gsz, classes)) for k in range(0, classes, gsz)
        ]
        with TileContext(nc) as tc:
            with tc.tile_pool(name="rawp", bufs=bufs) as rawp, \
                 tc.tile_pool(name="rowp", bufs=bufs) as rowp:

                def _body():
                    for c0, c1 in gbounds:
                        w = c1 - c0
                        raw = rawp.tile([128, w * 1024], f32, tag="raw")
                        row = rowp.tile([128, w * 1024], f32, tag="row")
                        raw_ap = raw[:]
                        for img in range(bpc):
                            xs = x[img].rearrange(
                                "(c u2) i j -> u2 c (i j)", u2=R * R
                            )
                            ring = (
                                ("sync", "scalar")[img]
                                if ring_split else load_ring
                            )
                            getattr(nc, ring).dma_start(
                                out=raw_ap[img * 64 : (img + 1) * 64],
                                in_=xs[:, c0:c1],
                            )
                        s_v = raw[:].rearrange(
                            "p (c i j) -> p c j i", c=w, i=H
                        )
                        d_v = row[:].rearrange(
                            "p (c r1l j r2) -> p c j r1l r2", c=w, r1l=4, r2=8
                        )
                        if merged_dve:
                            nc.vector.transpose(d_v, s_v)
                        else:
                            for cc in range(w):
                                nc.vector.transpose(d_v[:, cc], s_v[:, cc])
                        row_s = row[:].rearrange(
                            "(img s i) (c r1l w2) -> img s c i r1l w2",
                            img=bpc, s=2, c=w, r1l=4,
                        )
                        for cc in range(w):
                            for img in range(bpc):
                                dst = out[img, c0 + cc, 0:HR, 0:HR].rearrange(
                                    "(i s2 r1l) w2 -> s2 i r1l w2", s2=2, r1l=4
                                )
                                for s in range(2):
                                    # route swdge_stores of every class's 4
                                    # stores to the SWDGE (gpsimd) queue as
                                    # a 3rd concurrent stream; rotate by
                                    # class so ring leftovers stay balanced
                                    # and img-aligned (img0=sync/even
                                    # engines, img1=scalar/odd)
                                    if (2 * s + img + cc) % 4 < swdge_stores:
                                        ring = "gpsimd"
                                    elif ring_split:
                                        ring = ("sync", "scalar")[img]
                                    else:
                                        ring = store_ring
                                    getattr(nc, ring).dma_start(
                                        out=dst[s], in_=row_s[img, s, cc]
                                    )

                if loop_repeats > 1:
                    with tc.For_i(0, loop_repeats, 1):
                        _body()
                else:
                    _body()
        nc.compile()
        return nc

    if padded_store:
        # V2: store full 304-wide rows (pad cols pre-zeroed in SBUF) so each
        # (img, class) store is ONE DMA over a contiguous 304KB DRAM range
        # (256 rows x 1216B).  +19% store bytes for 2x fewer store DMAs with
        # 4.8x bigger descriptors.  Row slot layout per class per partition:
        # f = r1l*304 + j*8 + r2, f in [r1l*304+256, r1l*304+304) stays 0.
        with TileContext(nc) as tc:
            with tc.tile_pool(name="rawp", bufs=bufs) as rawp, \
                 tc.tile_pool(name="rowp", bufs=bufs) as rowp:

                def _body():
                    xs = x[:].rearrange(
                        "img (c u2) i j -> img u2 c (i j)", u2=R * R
                    )
                    for c in range(classes):
                        raw = rawp.tile([128, 1024], f32, tag="raw")
                        getattr(nc, load_ring).dma_start(
                            out=raw[:], in_=xs[:, :, c : c + 1]
                        )
                        row = rowp.tile([128, 4 * OUT], f32, tag="row")
                        rq = row[:].rearrange("p (r1l q) -> p r1l q", r1l=4)
                        # pad cols: tiny strided memset, on GpSimd so DVE
                        # and the HWDGE ring sequencers stay free
                        nc.gpsimd.memset(rq[:, :, HR:OUT], 0.0)
                        s_v = raw[:].rearrange("p (i j) -> p j i", i=H)
                        d_v = rq[:, :, 0:HR].rearrange(
                            "p r1l (j r2) -> p j r1l r2", r2=8
                        )
                        nc.vector.transpose(d_v, s_v)
                        src_v = row[:].rearrange(
                            "(img s2 i) f -> img i s2 f", img=bpc, s2=2
                        )
                        for img in range(bpc):
                            ring = (
                                ("scalar", "sync")[img]
                                if store_ring == "split"
                                else store_ring
                            )
                            getattr(nc, ring).dma_start(
                                out=out[img, c, 0:HR, :], in_=src_v[img]
                            )

                if loop_repeats > 1:
                    with tc.For_i(0, loop_repeats, 1):
                        _body()
                else:
                    _body()
        nc.compile()
        return nc

    with TileContext(nc) as tc:
        with tc.tile_pool(name="rawp", bufs=bufs) as rawp, \
             tc.tile_pool(name="rowp", bufs=bufs) as rowp:

          def _body():
            # DRAM-side load AP: (img, u2) together span the 128 partitions
            # (positional stream correspondence; shapes need not match).
            xs = x[:].rearrange("img (c u2) i j -> img u2 c (i j)", u2=R * R)
            for c0, c1 in bounds:
                w = c1 - c0
                raw = rawp.tile([128, w * 1024], f32, tag="raw")
                row = rowp.tile([128, w * 1024], f32, tag="row")
                # ---- load: one 128-partition DMA, 4KB DRAM descriptors ----
                getattr(nc, load_ring).dma_start(
                    out=raw[:], in_=xs[:, :, c0:c1]
                )
                # ---- shuffle: one DVE stream-transpose per class ----
                s_v = raw[:].rearrange("p (c i j) -> p c j i", c=w, i=H)
                d_v = row[:].rearrange(
                    "p (c r1l j r2) -> p c j r1l r2", c=w, r1l=4, r2=8
                )
                for cc in range(w):
                    nc.vector.transpose(d_v[:, cc], s_v[:, cc])
                # ---- store: one DMA per (class, img, s); 1KB DRAM rows ----
                # (3-dim DMA AP limit + the transpose's fixed partition
                # layout img*64+s*32+i make this the coarsest legal split.)
                row_s = row[:].rearrange(
                    "(img s i) (c r1l w2) -> img s c i r1l w2",
                    img=bpc, s=2, c=w, r1l=4,
                )
                for cc in range(w):
                    for img in range(bpc):
                        dst = out[img, c0 + cc, 0:HR, 0:HR].rearrange(
                            "(i s2 r1l) w2 -> s2 i r1l w2", s2=2, r1l=4
                        )
                        for s in range(2):
                            getattr(nc, store_ring).dma_start(
                                out=dst[s], in_=row_s[img, s, cc]
                            )

          if loop_repeats > 1:
              # measurement-only: on-device loop to amortize dispatch noise
              with tc.For_i(0, loop_repeats, 1):
                  _body()
          else:
              _body()
    nc.compile()
    return nc


def _get_nc():
    key = "main"
    if key not in _NC_CACHE:
        _NC_CACHE[key] = build_nc()
    return _NC_CACHE[key]


def kernel(x: np.ndarray) -> np.ndarray:
    from concourse.bass_utils import run_bass_kernel_spmd

    x = np.ascontiguousarray(x, dtype=np.float32)
    assert x.shape == (B, CLASSES * R * R, H, W), x.shape
    nc = _get_nc()
    in_maps = [{"x": x[k * BPC : (k + 1) * BPC]} for k in range(N_CORES)]
    res = run_bass_kernel_spmd(nc, in_maps, list(range(N_CORES)))
    return np.concatenate([res.results[k]["out"] for k in range(N_CORES)], axis=0)
